# revision 32
# baseline (speedup 1.0000x reference)
"""Trainium2 Bass kernel for nn_Attention_38130719654026 (sparse_attention).

The reference collapses exactly (verified to 9e-8 rel err): the top-k gather
broadcasts kv over the topk axis, so attention logits are constant along it,
softmax is uniform, and attn @ v_sel returns v unchanged.  grad/q/k are dead.
What remains is:

    g   = gelu(x @ W_v)              with W_v = W_qkv[:, 1024:1536]
    y[b, P, n, 64H:64H+64] = g[b, 2H + P//8, n, 64*(P%8) : 64*(P%8)+64]
    out = y @ W_out + b_out

Sharding (8 cores, no collectives): core c -> (batch b = c//4,
window parity q = (c//2)%2, token half t = c%2).  Each core computes the 8
output windows P in [8q, 8q+8) for its 128-token slice; these need exactly
the 8 source windows p = q + 2H at the same tokens, so x is read exactly
once across the fleet.

Raw bass (no TileContext: this walrus build rejects >1 embedded sync wait
per instruction, which Tile's kernel-tail drain always produces).  bf16
compute, fp32 PSUM accumulate, bf16 output (host casts back to fp32).

Pipeline per core (one PSUM bank per matmul group, all 8 banks used):
  8 input pieces split across BOTH hardware DGE rings (sync/SP ring:
  k0, k2, xtB-a, wo-a; scalar/ACT ring: k1, k3, xtB-b, wo-b) -- one ring
  alone sustains only ~250 GB/s vs ~300+ for two.  48 small dummy
  matmuls warm the PE HAM clock gate during the DMA; k-major mm1 starts
  after the first 0.25 MB piece lands.  Gelu on ScalarE (spline table
  pre-warmed), W_out halves shift-duplicated across partition halves by
  DVE (33-64ch ops may write either half), PSUM->SBUF output copies on
  DVE, 8 per-window output DMAs split across both rings.  mm2 emits
  window pairs interleaved: adjacent matmuls use PE row groups 0-1 vs
  2-3 (partition base 0 vs 64) and different PSUM banks, so they run
  concurrently (~107 ns per N=512 matmul); each pair's first four
  H-chunks gate only on the token-half-A gelu.
"""

import sys

sys.path.insert(0, "/opt/trn_rl_repo")

import numpy as np
import ml_dtypes

B, P_WIN, N_TOK, DIM = 2, 16, 256, 512
H_HEADS, DH = 8, 64
INNER = H_HEADS * DH  # 512
TOK_HALF = N_TOK // 2  # 128
N_CORES = 8

# Column layout of the packed input mega-tile (128 partitions, bf16):
#   [1024k : 1024k+512)    wv k-chunk;  [1024k+512 : 1024k+1024) xtA k-chunk
#   [4096:6144)   xtB : X^T token-half B, 4 k-chunks of (128, 512)
#   [6144:8192)   wo  : W_out natural, 4 tiles of (128, 512)
XTB_OFF, WO_OFF, IN_COLS = 4096, 6144, 8192

_COMPILED = None


def _build_bass():
    import concourse.bass as bass
    import concourse.mybir as mybir

    dt = mybir.dt
    nc = bass.Bass()

    inp = nc.declare_dram_parameter("inp", [128, IN_COLS], dt.bfloat16, isOutput=False)
    out = nc.declare_dram_parameter("out", [8, 128, 512], dt.bfloat16, isOutput=True)

    from contextlib import ExitStack

    stack = ExitStack()
    sem = lambda n: stack.enter_context(nc.semaphore(n))
    with (
        nc.sbuf_tensor([128, IN_COLS], dt.bfloat16) as mega,
        nc.sbuf_tensor([128, 4, 1024], dt.bfloat16) as g_t,
        nc.sbuf_tensor([128, 8, 512], dt.bfloat16) as out_t,
        nc.sbuf_tensor([128, 16], dt.float32) as scratch,
        nc.sbuf_tensor([128, 192], dt.bfloat16) as scratch_bf,
        nc.sbuf_tensor([128, 4, 512], dt.bfloat16) as wo_shift,
        nc.psum_tensor([128, 8, 512], dt.float32) as ps,
        stack,
        nc.Block() as block,
    ):
        ka0_sem, ka0b_sem = sem("ka0_sem"), sem("ka0b_sem")
        ka1_sem, ka2_sem, ka3_sem = sem("ka1_sem"), sem("ka2_sem"), sem("ka3_sem")
        xb1_sem, xb2_sem = sem("xb1_sem"), sem("xb2_sem")
        wo1_sem, wo2_sem = sem("wo1_sem"), sem("wo2_sem")
        dmao_sem = sem("dmao_sem")
        pe1_sem, pe2_sem = sem("pe1_sem"), sem("pe2_sem")
        act_sem, dve_sem = sem("act_sem"), sem("dve_sem")
        ka_sems = [ka0_sem, ka1_sem, ka2_sem, ka3_sem]

        def xt_sl(k, nch):
            off = (1024 * k + 512) if nch == 0 else (XTB_OFF + 512 * k)
            return mega[:, off : off + 512]

        def wv_sl(k, m):
            return mega[:, 1024 * k + 128 * m : 1024 * k + 128 * m + 128]

        def wo_sl(h, jp):
            if jp == 64 * (h % 2):  # natural position in the loaded W_out
                o = WO_OFF + 512 * (h // 2)
                return mega[jp : jp + 64, o : o + 512]
            return wo_shift[jp : jp + 64, h // 2, :]

        @block.sync
        def _(sync):
            mid = XTB_OFF + 1024
            wm = WO_OFF + 1024
            for k in (0, 2):
                o = 1024 * k
                sync.dma_start(out=mega[:, o : o + 1024], in_=inp[:, o : o + 1024]).then_inc(
                    ka_sems[k], 16
                )
            sync.dma_start(out=mega[:, XTB_OFF:mid], in_=inp[:, XTB_OFF:mid]).then_inc(xb1_sem, 16)
            sync.dma_start(out=mega[:, WO_OFF:wm], in_=inp[:, WO_OFF:wm]).then_inc(wo1_sem, 16)
            for pl in range(4):
                sync.wait_ge(dve_sem, pl + 7)
                sync.dma_start(out=out[pl], in_=out_t[:, pl, :]).then_inc(dmao_sem, 16)
            for pl in (6, 7):
                sync.wait_ge(dve_sem, 2 * pl + 1)  # lower-half copy done
                sync.dma_start(out=out[pl, 0:64, :], in_=out_t[0:64, pl, :]).then_inc(dmao_sem, 16)
            sync.wait_ge(dmao_sem, 160)

        @block.tensor
        def _(tensor):
            # Warm the PE HAM clock with small dummy matmuls during the DMA.
            tensor.wait_ge(dve_sem, 2)
            for _ in range(48):
                nc.tensor.matmul(
                    ps[:, 0, 0:64],
                    lhsT=scratch_bf[:, 0:128],
                    rhs=scratch_bf[:, 128:192],
                    start=True,
                    stop=True,
                    skip_group_check=True,
                )
            # mm1 (k-major): gT = W_v^T @ X^T; group (m, nch) -> bank 4*nch+m
            for nch in range(2):
                for k in range(4):
                    if nch == 0:
                        tensor.wait_ge(ka_sems[k], 16)
                    elif k == 0:
                        tensor.wait_ge(xb1_sem, 16)
                    elif k == 2:
                        tensor.wait_ge(xb2_sem, 16)
                    for m in range(4):
                        mm = nc.tensor.matmul(
                            ps[:, 4 * nch + m, :],
                            lhsT=wv_sl(k, m),
                            rhs=xt_sl(k, nch),
                            start=(k == 0),
                            stop=(k == 3),
                            skip_group_check=True,
                        )
                        if k == 3:
                            mm.then_inc(pe1_sem, 1)
            # mm2: window pairs (2pp, 2pp+1) interleaved across PE row groups;
            # window pl -> bank pl (freed by gelu pl before act_sem >= 5+pp)
            for pp in range(4):
                tensor.wait_ge(act_sem, 2 * pp + 2)  # banks freed + half A
                if pp == 0:
                    tensor.wait_ge(dve_sem, 4)  # wo_shift H0-3 built
                for hh in range(8):
                    if hh == 4:
                        tensor.wait_ge(act_sem, 5 + pp)  # g tile pp, half B
                        if pp == 0:
                            tensor.wait_ge(dve_sem, 6)  # wo_shift H4-7 built
                    for pl in (2 * pp, 2 * pp + 1):
                        jp = 64 * (pl % 2)
                        mm = nc.tensor.matmul(
                            ps[:, pl, :],
                            lhsT=g_t[jp : jp + 64, pp, 128 * hh : 128 * hh + 128],
                            rhs=wo_sl(hh, jp),
                            start=(hh == 0),
                            stop=(hh == 7),
                            skip_group_check=True,
                        )
                        if hh == 7:
                            mm.then_inc(pe2_sem, 1)

        @block.scalar
        def _(scalar):
            # Second HWDGE ring (qActDynamicHW): half the input stream runs
            # here concurrently with the sync ring.
            mid = XTB_OFF + 1024
            wm = WO_OFF + 1024
            for k in (1, 3):
                o = 1024 * k
                scalar.dma_start(out=mega[:, o : o + 1024], in_=inp[:, o : o + 1024]).then_inc(
                    ka_sems[k], 16
                )
            scalar.dma_start(out=mega[:, mid:WO_OFF], in_=inp[:, mid:WO_OFF]).then_inc(xb2_sem, 16)
            scalar.dma_start(out=mega[:, wm:], in_=inp[:, wm:]).then_inc(wo2_sem, 16)
            # Pre-warm the gelu spline table during the input DMA.
            scalar.wait_ge(dve_sem, 1)
            nc.scalar.activation(
                scratch[:, 8:], scratch[:, :8], mybir.ActivationFunctionType.Gelu
            )
            for i in range(8):
                m, nch = i % 4, i // 4
                scalar.wait_ge(pe1_sem, i + 1)
                nc.scalar.activation(
                    g_t[:, m, 512 * nch : 512 * nch + 512],
                    ps[:, 4 * nch + m, :],
                    mybir.ActivationFunctionType.Gelu,
                ).then_inc(act_sem, 1)
            for pl in (4, 5):
                scalar.wait_ge(dve_sem, pl + 7)
                scalar.dma_start(out=out[pl], in_=out_t[:, pl, :]).then_inc(dmao_sem, 16)
            for pl in (6, 7):
                scalar.wait_ge(dve_sem, 2 * pl + 2)  # upper-half copy done
                scalar.dma_start(out=out[pl, 64:128, :], in_=out_t[64:128, pl, :]).then_inc(dmao_sem, 16)

        @block.vector
        def _(vector):
            nc.vector.memset(scratch[:, :8], 0.0).then_inc(dve_sem, 1)
            nc.vector.memset(scratch_bf[:], 0.0).then_inc(dve_sem, 1)
            # Build wo_shift: each W_out row-half copied to the OPPOSITE
            # partition half (DVE 33-64ch ops may write either half); the
            # matching half is read from mega directly.
            for piece, wsem in ((0, wo1_sem), (1, wo2_sem)):
                vector.wait_ge(wsem, 16)
                o = WO_OFF + 1024 * piece
                src_lo = mega[0:64, o : o + 1024].rearrange("p (c t) -> p c t", c=2)
                src_hi = mega[64:128, o : o + 1024].rearrange("p (c t) -> p c t", c=2)
                d0, d1 = 2 * piece, 2 * piece + 2
                nc.vector.tensor_copy(wo_shift[64:128, d0:d1, :], src_lo).then_inc(dve_sem, 1)
                nc.vector.tensor_copy(wo_shift[0:64, d0:d1, :], src_hi).then_inc(dve_sem, 1)
            for pl in range(6):
                vector.wait_ge(pe2_sem, pl + 1)
                nc.vector.tensor_copy(out_t[:, pl, :], ps[:, pl, :]).then_inc(
                    dve_sem, 1
                )
            # last pair: half-partition copies so the two output DMAs per
            # window can start as early as possible on both rings
            for pl in (6, 7):
                vector.wait_ge(pe2_sem, pl + 1)
                nc.vector.tensor_copy(out_t[0:64, pl, :], ps[0:64, pl, :]).then_inc(
                    dve_sem, 1
                )
                nc.vector.tensor_copy(out_t[64:128, pl, :], ps[64:128, pl, :]).then_inc(
                    dve_sem, 1
                )

    return nc


def _shard_inputs(x, W_qkv, W_out):
    bf16 = ml_dtypes.bfloat16
    W_v = np.ascontiguousarray(W_qkv[:, 2 * INNER : 3 * INNER]).astype(bf16)
    wv_chunks = W_v.reshape(4, 128, 512).transpose(1, 0, 2)  # (128, 4, 512)
    wo_part = (
        W_out.astype(bf16).reshape(4, 128, 512).transpose(1, 0, 2).reshape(128, 2048)
    )
    in_maps = []
    for c in range(N_CORES):
        b, q, t = c // 4, (c // 2) % 2, c % 2
        xs = x[b, q::2, TOK_HALF * t : TOK_HALF * (t + 1), :]  # (8, 128, 512)
        xt = np.ascontiguousarray(xs.transpose(2, 0, 1).reshape(512, 1024)).astype(bf16)
        xt4 = xt.reshape(4, 128, 1024)
        xtA = xt4[:, :, :512].transpose(1, 0, 2)  # (128, 4, 512)
        xtB = xt4[:, :, 512:].transpose(1, 0, 2).reshape(128, 2048)
        front = np.concatenate([wv_chunks, xtA], axis=2).reshape(128, 4096)
        mega = np.concatenate([front, xtB, wo_part], axis=1)
        in_maps.append({"inp": np.ascontiguousarray(mega)})
    return in_maps


def _assemble(results, b_out):
    out = np.empty((B, P_WIN, N_TOK, DIM), dtype=np.float32)
    for c in range(N_CORES):
        b, q, t = c // 4, (c // 2) % 2, c % 2
        r = np.asarray(results[c]["out"]).astype(np.float32)  # (8, 128, 512)
        out[b, 8 * q : 8 * q + 8, TOK_HALF * t : TOK_HALF * (t + 1), :] = r
    out += b_out.astype(np.float32)
    return out


def _run(inputs, trace=False, trace_cores=None):
    global _COMPILED
    from concourse.bass_utils import run_bass_kernel_spmd

    if _COMPILED is None:
        _COMPILED = _build_bass()
    nc = _COMPILED
    in_maps = _shard_inputs(
        np.asarray(inputs["x"]), np.asarray(inputs["W_qkv"]), np.asarray(inputs["W_out"])
    )
    res = run_bass_kernel_spmd(
        nc, in_maps, core_ids=list(range(N_CORES)), trace=trace, trace_cores=trace_cores
    )
    out = _assemble(res.results, np.asarray(inputs["b_out"]))
    return out, res


def kernel(x, grad, W_qkv, W_out, b_out):
    out, _ = _run(dict(x=x, grad=grad, W_qkv=W_qkv, W_out=W_out, b_out=b_out))
    return out


# revision 33
# speedup vs baseline: 1.0556x; 1.0556x over previous
"""Trainium2 Bass kernel for nn_Attention_38130719654026 (sparse_attention).

The reference collapses exactly (verified to 9e-8 rel err): the top-k gather
broadcasts kv over the topk axis, so attention logits are constant along it,
softmax is uniform, and attn @ v_sel returns v unchanged.  grad/q/k are dead.
What remains is:

    g   = gelu(x @ W_v)              with W_v = W_qkv[:, 1024:1536]
    y[b, P, n, 64H:64H+64] = g[b, 2H + P//8, n, 64*(P%8) : 64*(P%8)+64]
    out = y @ W_out + b_out

Sharding (8 cores, no collectives): core c -> (batch b = c//4,
window parity q = (c//2)%2, token half t = c%2).  Each core computes the 8
output windows P in [8q, 8q+8) for its 128-token slice; these need exactly
the 8 source windows p = q + 2H at the same tokens, so x is read exactly
once across the fleet.

Raw bass (no TileContext: this walrus build rejects >1 embedded sync wait
per instruction, which Tile's kernel-tail drain always produces).  bf16
compute, fp32 PSUM accumulate, bf16 output (host casts back to fp32).

Pipeline per core (one PSUM bank per matmul group, all 8 banks used):
  8 input pieces split across BOTH hardware DGE rings (sync/SP ring:
  k0, k2, xtB-a, wo-a; scalar/ACT ring: k1, k3, xtB-b, wo-b) -- one ring
  alone sustains only ~250 GB/s vs ~300+ for two.  48 small dummy
  matmuls warm the PE HAM clock gate during the DMA; k-major mm1 starts
  after the first 0.25 MB piece lands.  Gelu on ScalarE (spline table
  pre-warmed), W_out halves shift-duplicated across partition halves by
  DVE (33-64ch ops may write either half), PSUM->SBUF output copies on
  DVE, 8 per-window output DMAs split across both rings.  mm2 emits
  window pairs interleaved: adjacent matmuls use PE row groups 0-1 vs
  2-3 (partition base 0 vs 64) and different PSUM banks, so they run
  concurrently (~107 ns per N=512 matmul); each pair's first four
  H-chunks gate only on the token-half-A gelu.
"""

import sys

sys.path.insert(0, "/opt/trn_rl_repo")

import numpy as np
import ml_dtypes

B, P_WIN, N_TOK, DIM = 2, 16, 256, 512
H_HEADS, DH = 8, 64
INNER = H_HEADS * DH  # 512
TOK_HALF = N_TOK // 2  # 128
N_CORES = 8

# Column layout of the packed input mega-tile (128 partitions, bf16):
#   [1024k : 1024k+512)    wv k-chunk;  [1024k+512 : 1024k+1024) xtA k-chunk
#   [4096:6144)   xtB : X^T token-half B, 4 k-chunks of (128, 512)
#   [6144:8192)   wo  : W_out natural, 4 tiles of (128, 512)
XTB_OFF, WO_OFF, IN_COLS = 4096, 6144, 8192

_COMPILED = None


def _build_bass():
    import concourse.bass as bass
    import concourse.mybir as mybir

    dt = mybir.dt
    nc = bass.Bass()

    inp = nc.declare_dram_parameter("inp", [128, IN_COLS], dt.bfloat16, isOutput=False)
    out = nc.declare_dram_parameter("out", [8, 128, 512], dt.bfloat16, isOutput=True)

    from contextlib import ExitStack

    stack = ExitStack()
    sem = lambda n: stack.enter_context(nc.semaphore(n))
    with (
        nc.sbuf_tensor([128, IN_COLS], dt.bfloat16) as mega,
        nc.sbuf_tensor([128, 4, 1024], dt.bfloat16) as g_t,
        nc.sbuf_tensor([128, 8, 512], dt.bfloat16) as out_t,
        nc.sbuf_tensor([128, 16], dt.float32) as scratch,
        nc.sbuf_tensor([128, 192], dt.bfloat16) as scratch_bf,
        nc.sbuf_tensor([128, 4, 512], dt.bfloat16) as wo_shift,
        nc.psum_tensor([128, 8, 512], dt.float32) as ps,
        stack,
        nc.Block() as block,
    ):
        ka0_sem, ka0b_sem = sem("ka0_sem"), sem("ka0b_sem")
        ka1_sem, ka2_sem, ka3_sem = sem("ka1_sem"), sem("ka2_sem"), sem("ka3_sem")
        xb1_sem, xb2_sem = sem("xb1_sem"), sem("xb2_sem")
        wo1_sem, wo2_sem = sem("wo1_sem"), sem("wo2_sem")
        dmao_sem = sem("dmao_sem")
        pe1_sem, pe2_sem = sem("pe1_sem"), sem("pe2_sem")
        act_sem, dve_sem = sem("act_sem"), sem("dve_sem")
        ka_sems = [ka0_sem, ka1_sem, ka2_sem, ka3_sem]

        def xt_sl(k, nch):
            off = (1024 * k + 512) if nch == 0 else (XTB_OFF + 512 * k)
            return mega[:, off : off + 512]

        def wv_sl(k, m):
            return mega[:, 1024 * k + 128 * m : 1024 * k + 128 * m + 128]

        def wo_sl(h, jp):
            if jp == 64 * (h % 2):  # natural position in the loaded W_out
                o = WO_OFF + 512 * (h // 2)
                return mega[jp : jp + 64, o : o + 512]
            return wo_shift[jp : jp + 64, h // 2, :]

        @block.sync
        def _(sync):
            mid = XTB_OFF + 1024
            wm = WO_OFF + 1024
            for k in (0, 2):
                o = 1024 * k
                sync.dma_start(out=mega[:, o : o + 1024], in_=inp[:, o : o + 1024]).then_inc(
                    ka_sems[k], 16
                )
            sync.dma_start(out=mega[:, XTB_OFF:mid], in_=inp[:, XTB_OFF:mid]).then_inc(xb1_sem, 16)
            sync.dma_start(out=mega[:, WO_OFF:wm], in_=inp[:, WO_OFF:wm]).then_inc(wo1_sem, 16)
            for pl in range(4):
                sync.wait_ge(dve_sem, pl + 7)
                sync.dma_start(out=out[pl], in_=out_t[:, pl, :]).then_inc(dmao_sem, 16)
            sync.wait_ge(dmao_sem, 128)

        @block.tensor
        def _(tensor):
            # Warm the PE HAM clock with small dummy matmuls during the DMA.
            tensor.wait_ge(dve_sem, 2)
            for _ in range(48):
                nc.tensor.matmul(
                    ps[:, 0, 0:64],
                    lhsT=scratch_bf[:, 0:128],
                    rhs=scratch_bf[:, 128:192],
                    start=True,
                    stop=True,
                    skip_group_check=True,
                )
            # mm1 (k-major): gT = W_v^T @ X^T; group (m, nch) -> bank 4*nch+m
            for nch in range(2):
                for k in range(4):
                    if nch == 0:
                        tensor.wait_ge(ka_sems[k], 16)
                    elif k == 0:
                        tensor.wait_ge(xb1_sem, 16)
                    elif k == 2:
                        tensor.wait_ge(xb2_sem, 16)
                    for m in range(4):
                        mm = nc.tensor.matmul(
                            ps[:, 4 * nch + m, :],
                            lhsT=wv_sl(k, m),
                            rhs=xt_sl(k, nch),
                            start=(k == 0),
                            stop=(k == 3),
                            skip_group_check=True,
                        )
                        if k == 3:
                            mm.then_inc(pe1_sem, 1)
            # mm2: window pairs (2pp, 2pp+1) interleaved across PE row groups;
            # window pl -> bank pl (freed by gelu pl before act_sem >= 5+pp)
            for pp in range(4):
                tensor.wait_ge(act_sem, 2 * pp + 2)  # banks freed + half A
                if pp == 0:
                    tensor.wait_ge(dve_sem, 4)  # wo_shift H0-3 built
                for hh in range(8):
                    if hh == 4:
                        tensor.wait_ge(act_sem, 5 + pp)  # g tile pp, half B
                        if pp == 0:
                            tensor.wait_ge(dve_sem, 6)  # wo_shift H4-7 built
                    for pl in (2 * pp, 2 * pp + 1):
                        jp = 64 * (pl % 2)
                        mm = nc.tensor.matmul(
                            ps[:, pl, :],
                            lhsT=g_t[jp : jp + 64, pp, 128 * hh : 128 * hh + 128],
                            rhs=wo_sl(hh, jp),
                            start=(hh == 0),
                            stop=(hh == 7),
                            skip_group_check=True,
                        )
                        if hh == 7:
                            mm.then_inc(pe2_sem, 1)

        @block.scalar
        def _(scalar):
            # Second HWDGE ring (qActDynamicHW): half the input stream runs
            # here concurrently with the sync ring.
            mid = XTB_OFF + 1024
            wm = WO_OFF + 1024
            for k in (1, 3):
                o = 1024 * k
                scalar.dma_start(out=mega[:, o : o + 1024], in_=inp[:, o : o + 1024]).then_inc(
                    ka_sems[k], 16
                )
            scalar.dma_start(out=mega[:, mid:WO_OFF], in_=inp[:, mid:WO_OFF]).then_inc(xb2_sem, 16)
            scalar.dma_start(out=mega[:, wm:], in_=inp[:, wm:]).then_inc(wo2_sem, 16)
            # Pre-warm the gelu spline table during the input DMA.
            scalar.wait_ge(dve_sem, 1)
            nc.scalar.activation(
                scratch[:, 8:], scratch[:, :8], mybir.ActivationFunctionType.Gelu
            )
            for i in range(8):
                m, nch = i % 4, i // 4
                scalar.wait_ge(pe1_sem, i + 1)
                nc.scalar.activation(
                    g_t[:, m, 512 * nch : 512 * nch + 512],
                    ps[:, 4 * nch + m, :],
                    mybir.ActivationFunctionType.Gelu,
                ).then_inc(act_sem, 1)
            for pl in range(4, 8):
                scalar.wait_ge(dve_sem, pl + 7)
                scalar.dma_start(out=out[pl], in_=out_t[:, pl, :]).then_inc(dmao_sem, 16)

        @block.vector
        def _(vector):
            nc.vector.memset(scratch[:, :8], 0.0).then_inc(dve_sem, 1)
            nc.vector.memset(scratch_bf[:], 0.0).then_inc(dve_sem, 1)
            # Build wo_shift: each W_out row-half copied to the OPPOSITE
            # partition half (DVE 33-64ch ops may write either half); the
            # matching half is read from mega directly.
            for piece, wsem in ((0, wo1_sem), (1, wo2_sem)):
                vector.wait_ge(wsem, 16)
                o = WO_OFF + 1024 * piece
                src_lo = mega[0:64, o : o + 1024].rearrange("p (c t) -> p c t", c=2)
                src_hi = mega[64:128, o : o + 1024].rearrange("p (c t) -> p c t", c=2)
                d0, d1 = 2 * piece, 2 * piece + 2
                nc.vector.tensor_copy(wo_shift[64:128, d0:d1, :], src_lo).then_inc(dve_sem, 1)
                nc.vector.tensor_copy(wo_shift[0:64, d0:d1, :], src_hi).then_inc(dve_sem, 1)
            for pl in range(8):
                vector.wait_ge(pe2_sem, pl + 1)
                nc.vector.tensor_copy(out_t[:, pl, :], ps[:, pl, :]).then_inc(
                    dve_sem, 1
                )

    return nc


def _shard_inputs(x, W_qkv, W_out):
    bf16 = ml_dtypes.bfloat16
    W_v = np.ascontiguousarray(W_qkv[:, 2 * INNER : 3 * INNER]).astype(bf16)
    wv_chunks = W_v.reshape(4, 128, 512).transpose(1, 0, 2)  # (128, 4, 512)
    wo_part = (
        W_out.astype(bf16).reshape(4, 128, 512).transpose(1, 0, 2).reshape(128, 2048)
    )
    in_maps = []
    for c in range(N_CORES):
        b, q, t = c // 4, (c // 2) % 2, c % 2
        xs = x[b, q::2, TOK_HALF * t : TOK_HALF * (t + 1), :]  # (8, 128, 512)
        xt = np.ascontiguousarray(xs.transpose(2, 0, 1).reshape(512, 1024)).astype(bf16)
        xt4 = xt.reshape(4, 128, 1024)
        xtA = xt4[:, :, :512].transpose(1, 0, 2)  # (128, 4, 512)
        xtB = xt4[:, :, 512:].transpose(1, 0, 2).reshape(128, 2048)
        front = np.concatenate([wv_chunks, xtA], axis=2).reshape(128, 4096)
        mega = np.concatenate([front, xtB, wo_part], axis=1)
        in_maps.append({"inp": np.ascontiguousarray(mega)})
    return in_maps


def _assemble(results, b_out):
    out = np.empty((B, P_WIN, N_TOK, DIM), dtype=np.float32)
    for c in range(N_CORES):
        b, q, t = c // 4, (c // 2) % 2, c % 2
        r = np.asarray(results[c]["out"]).astype(np.float32)  # (8, 128, 512)
        out[b, 8 * q : 8 * q + 8, TOK_HALF * t : TOK_HALF * (t + 1), :] = r
    out += b_out.astype(np.float32)
    return out


def _run(inputs, trace=False, trace_cores=None):
    global _COMPILED
    from concourse.bass_utils import run_bass_kernel_spmd

    if _COMPILED is None:
        _COMPILED = _build_bass()
    nc = _COMPILED
    in_maps = _shard_inputs(
        np.asarray(inputs["x"]), np.asarray(inputs["W_qkv"]), np.asarray(inputs["W_out"])
    )
    res = run_bass_kernel_spmd(
        nc, in_maps, core_ids=list(range(N_CORES)), trace=trace, trace_cores=trace_cores
    )
    out = _assemble(res.results, np.asarray(inputs["b_out"]))
    return out, res


def kernel(x, grad, W_qkv, W_out, b_out):
    out, _ = _run(dict(x=x, grad=grad, W_qkv=W_qkv, W_out=W_out, b_out=b_out))
    return out


# revision 34
# speedup vs baseline: 1.0813x; 1.0243x over previous
"""Trainium2 Bass kernel for nn_Attention_38130719654026 (sparse_attention).

The reference collapses exactly (verified to 9e-8 rel err): the top-k gather
broadcasts kv over the topk axis, so attention logits are constant along it,
softmax is uniform, and attn @ v_sel returns v unchanged.  grad/q/k are dead.
What remains is:

    g   = gelu(x @ W_v)              with W_v = W_qkv[:, 1024:1536]
    y[b, P, n, 64H:64H+64] = g[b, 2H + P//8, n, 64*(P%8) : 64*(P%8)+64]
    out = y @ W_out + b_out

Sharding (8 cores, no collectives): core c -> (batch b = c//4,
window parity q = (c//2)%2, token half t = c%2).  Each core computes the 8
output windows P in [8q, 8q+8) for its 128-token slice; these need exactly
the 8 source windows p = q + 2H at the same tokens, so x is read exactly
once across the fleet.

Raw bass (no TileContext: this walrus build rejects >1 embedded sync wait
per instruction, which Tile's kernel-tail drain always produces).  bf16
compute, fp32 PSUM accumulate, bf16 output (host casts back to fp32).

Pipeline per core (one PSUM bank per matmul group, all 8 banks used):
  8 input pieces split across BOTH hardware DGE rings (sync/SP ring:
  k0, k2, xtB-a, wo-a; scalar/ACT ring: k1, k3, xtB-b, wo-b) -- one ring
  alone sustains only ~250 GB/s vs ~300+ for two.  48 small dummy
  matmuls warm the PE HAM clock gate during the DMA; k-major mm1 starts
  after the first 0.25 MB piece lands.  Gelu on ScalarE (spline table
  pre-warmed), W_out halves shift-duplicated across partition halves by
  DVE (33-64ch ops may write either half), PSUM->SBUF output copies on
  DVE, 8 per-window output DMAs split across both rings.  mm2 emits
  window pairs interleaved: adjacent matmuls use PE row groups 0-1 vs
  2-3 (partition base 0 vs 64) and different PSUM banks, so they run
  concurrently (~107 ns per N=512 matmul); each pair's first four
  H-chunks gate only on the token-half-A gelu.
"""

import sys

sys.path.insert(0, "/opt/trn_rl_repo")

import numpy as np
import ml_dtypes

B, P_WIN, N_TOK, DIM = 2, 16, 256, 512
H_HEADS, DH = 8, 64
INNER = H_HEADS * DH  # 512
TOK_HALF = N_TOK // 2  # 128
N_CORES = 8

# Column layout of the packed input mega-tile (128 partitions, bf16):
#   [1024k : 1024k+512)    wv k-chunk;  [1024k+512 : 1024k+1024) xtA k-chunk
#   [4096:6144)   xtB : X^T token-half B, 4 k-chunks of (128, 512)
#   [6144:8192)   wo  : W_out natural, 4 tiles of (128, 512)
XTB_OFF, WO_OFF, IN_COLS = 4096, 6144, 8192

_COMPILED = None


def _build_bass():
    import concourse.bass as bass
    import concourse.mybir as mybir

    dt = mybir.dt
    nc = bass.Bass()

    inp = nc.declare_dram_parameter("inp", [128, IN_COLS], dt.bfloat16, isOutput=False)
    out = nc.declare_dram_parameter("out", [8, 128, 512], dt.bfloat16, isOutput=True)

    from contextlib import ExitStack

    stack = ExitStack()
    sem = lambda n: stack.enter_context(nc.semaphore(n))
    with (
        nc.sbuf_tensor([128, IN_COLS], dt.bfloat16) as mega,
        nc.sbuf_tensor([128, 4, 1024], dt.bfloat16) as g_t,
        nc.sbuf_tensor([128, 8, 512], dt.bfloat16) as out_t,
        nc.sbuf_tensor([128, 16], dt.float32) as scratch,
        nc.sbuf_tensor([128, 192], dt.bfloat16) as scratch_bf,
        nc.sbuf_tensor([128, 4, 512], dt.bfloat16) as wo_shift,
        nc.psum_tensor([128, 8, 512], dt.float32) as ps,
        stack,
        nc.Block() as block,
    ):
        ka0_sem, ka0b_sem = sem("ka0_sem"), sem("ka0b_sem")
        ka1_sem, ka2_sem, ka3_sem = sem("ka1_sem"), sem("ka2_sem"), sem("ka3_sem")
        xb1_sem, xb2_sem = sem("xb1_sem"), sem("xb2_sem")
        wo1_sem, wo2_sem = sem("wo1_sem"), sem("wo2_sem")
        dmao_sem = sem("dmao_sem")
        pe1_sem, pe2_sem = sem("pe1_sem"), sem("pe2_sem")
        act_sem, dve_sem = sem("act_sem"), sem("dve_sem")
        ka_sems = [ka0_sem, ka1_sem, ka2_sem, ka3_sem]

        def xt_sl(k, nch):
            off = (1024 * k + 512) if nch == 0 else (XTB_OFF + 512 * k)
            return mega[:, off : off + 512]

        def wv_sl(k, m):
            return mega[:, 1024 * k + 128 * m : 1024 * k + 128 * m + 128]

        def wo_sl(h, jp):
            if jp == 64 * (h % 2):  # natural position in the loaded W_out
                o = WO_OFF + 512 * (h // 2)
                return mega[jp : jp + 64, o : o + 512]
            return wo_shift[jp : jp + 64, h // 2, :]

        @block.sync
        def _(sync):
            mid = XTB_OFF + 1024
            wm = WO_OFF + 1024
            for k in (0, 2):
                o = 1024 * k
                sync.dma_start(out=mega[:, o : o + 1024], in_=inp[:, o : o + 1024]).then_inc(
                    ka_sems[k], 16
                )
            sync.dma_start(out=mega[:, XTB_OFF:mid], in_=inp[:, XTB_OFF:mid]).then_inc(xb1_sem, 16)
            sync.dma_start(out=mega[:, WO_OFF:wm], in_=inp[:, WO_OFF:wm]).then_inc(wo1_sem, 16)
            for pl in range(4):
                sync.wait_ge(dve_sem, pl + 7)
                sync.dma_start(out=out[pl], in_=out_t[:, pl, :]).then_inc(dmao_sem, 16)
            sync.wait_ge(dmao_sem, 128)

        @block.tensor
        def _(tensor):
            # Warm the PE HAM clock with small dummy matmuls during the DMA.
            tensor.wait_ge(dve_sem, 2)
            for _ in range(52):
                nc.tensor.matmul(
                    ps[:, 0, 0:64],
                    lhsT=scratch_bf[:, 0:128],
                    rhs=scratch_bf[:, 128:192],
                    start=True,
                    stop=True,
                    skip_group_check=True,
                )
            # mm1 (k-major): gT = W_v^T @ X^T; group (m, nch) -> bank 4*nch+m
            for nch in range(2):
                for k in range(4):
                    if nch == 0:
                        tensor.wait_ge(ka_sems[k], 16)
                    elif k == 0:
                        tensor.wait_ge(xb1_sem, 16)
                    elif k == 2:
                        tensor.wait_ge(xb2_sem, 16)
                    for m in range(4):
                        mm = nc.tensor.matmul(
                            ps[:, 4 * nch + m, :],
                            lhsT=wv_sl(k, m),
                            rhs=xt_sl(k, nch),
                            start=(k == 0),
                            stop=(k == 3),
                            skip_group_check=True,
                        )
                        if k == 3:
                            mm.then_inc(pe1_sem, 1)
            # mm2: window pairs (2pp, 2pp+1) interleaved across PE row groups;
            # window pl -> bank pl (freed by gelu pl before act_sem >= 5+pp)
            for pp in range(4):
                tensor.wait_ge(act_sem, 2 * pp + 2)  # banks freed + half A
                if pp == 0:
                    tensor.wait_ge(dve_sem, 4)  # wo_shift H0-3 built
                for hh in range(8):
                    if hh == 4:
                        tensor.wait_ge(act_sem, 5 + pp)  # g tile pp, half B
                        if pp == 0:
                            tensor.wait_ge(dve_sem, 6)  # wo_shift H4-7 built
                    for pl in (2 * pp, 2 * pp + 1):
                        jp = 64 * (pl % 2)
                        mm = nc.tensor.matmul(
                            ps[:, pl, :],
                            lhsT=g_t[jp : jp + 64, pp, 128 * hh : 128 * hh + 128],
                            rhs=wo_sl(hh, jp),
                            start=(hh == 0),
                            stop=(hh == 7),
                            skip_group_check=True,
                        )
                        if hh == 7:
                            mm.then_inc(pe2_sem, 1)

        @block.scalar
        def _(scalar):
            # Second HWDGE ring (qActDynamicHW): half the input stream runs
            # here concurrently with the sync ring.
            mid = XTB_OFF + 1024
            wm = WO_OFF + 1024
            for k in (1, 3):
                o = 1024 * k
                scalar.dma_start(out=mega[:, o : o + 1024], in_=inp[:, o : o + 1024]).then_inc(
                    ka_sems[k], 16
                )
            scalar.dma_start(out=mega[:, mid:WO_OFF], in_=inp[:, mid:WO_OFF]).then_inc(xb2_sem, 16)
            scalar.dma_start(out=mega[:, wm:], in_=inp[:, wm:]).then_inc(wo2_sem, 16)
            # Pre-warm the gelu spline table during the input DMA.
            scalar.wait_ge(dve_sem, 1)
            nc.scalar.activation(
                scratch[:, 8:], scratch[:, :8], mybir.ActivationFunctionType.Gelu
            )
            for i in range(8):
                m, nch = i % 4, i // 4
                scalar.wait_ge(pe1_sem, i + 1)
                nc.scalar.activation(
                    g_t[:, m, 512 * nch : 512 * nch + 512],
                    ps[:, 4 * nch + m, :],
                    mybir.ActivationFunctionType.Gelu,
                ).then_inc(act_sem, 1)
            for pl in range(4, 8):
                scalar.wait_ge(dve_sem, pl + 7)
                scalar.dma_start(out=out[pl], in_=out_t[:, pl, :]).then_inc(dmao_sem, 16)

        @block.vector
        def _(vector):
            nc.vector.memset(scratch[:, :8], 0.0).then_inc(dve_sem, 1)
            nc.vector.memset(scratch_bf[:], 0.0).then_inc(dve_sem, 1)
            # Build wo_shift: each W_out row-half copied to the OPPOSITE
            # partition half (DVE 33-64ch ops may write either half); the
            # matching half is read from mega directly.
            for piece, wsem in ((0, wo1_sem), (1, wo2_sem)):
                vector.wait_ge(wsem, 16)
                o = WO_OFF + 1024 * piece
                src_lo = mega[0:64, o : o + 1024].rearrange("p (c t) -> p c t", c=2)
                src_hi = mega[64:128, o : o + 1024].rearrange("p (c t) -> p c t", c=2)
                d0, d1 = 2 * piece, 2 * piece + 2
                nc.vector.tensor_copy(wo_shift[64:128, d0:d1, :], src_lo).then_inc(dve_sem, 1)
                nc.vector.tensor_copy(wo_shift[0:64, d0:d1, :], src_hi).then_inc(dve_sem, 1)
            for pl in range(8):
                vector.wait_ge(pe2_sem, pl + 1)
                nc.vector.tensor_copy(out_t[:, pl, :], ps[:, pl, :]).then_inc(
                    dve_sem, 1
                )

    return nc


def _shard_inputs(x, W_qkv, W_out):
    bf16 = ml_dtypes.bfloat16
    W_v = np.ascontiguousarray(W_qkv[:, 2 * INNER : 3 * INNER]).astype(bf16)
    wv_chunks = W_v.reshape(4, 128, 512).transpose(1, 0, 2)  # (128, 4, 512)
    wo_part = (
        W_out.astype(bf16).reshape(4, 128, 512).transpose(1, 0, 2).reshape(128, 2048)
    )
    in_maps = []
    for c in range(N_CORES):
        b, q, t = c // 4, (c // 2) % 2, c % 2
        xs = x[b, q::2, TOK_HALF * t : TOK_HALF * (t + 1), :]  # (8, 128, 512)
        xt = np.ascontiguousarray(xs.transpose(2, 0, 1).reshape(512, 1024)).astype(bf16)
        xt4 = xt.reshape(4, 128, 1024)
        xtA = xt4[:, :, :512].transpose(1, 0, 2)  # (128, 4, 512)
        xtB = xt4[:, :, 512:].transpose(1, 0, 2).reshape(128, 2048)
        front = np.concatenate([wv_chunks, xtA], axis=2).reshape(128, 4096)
        mega = np.concatenate([front, xtB, wo_part], axis=1)
        in_maps.append({"inp": np.ascontiguousarray(mega)})
    return in_maps


def _assemble(results, b_out):
    out = np.empty((B, P_WIN, N_TOK, DIM), dtype=np.float32)
    for c in range(N_CORES):
        b, q, t = c // 4, (c // 2) % 2, c % 2
        r = np.asarray(results[c]["out"]).astype(np.float32)  # (8, 128, 512)
        out[b, 8 * q : 8 * q + 8, TOK_HALF * t : TOK_HALF * (t + 1), :] = r
    out += b_out.astype(np.float32)
    return out


def _run(inputs, trace=False, trace_cores=None):
    global _COMPILED
    from concourse.bass_utils import run_bass_kernel_spmd

    if _COMPILED is None:
        _COMPILED = _build_bass()
    nc = _COMPILED
    in_maps = _shard_inputs(
        np.asarray(inputs["x"]), np.asarray(inputs["W_qkv"]), np.asarray(inputs["W_out"])
    )
    res = run_bass_kernel_spmd(
        nc, in_maps, core_ids=list(range(N_CORES)), trace=trace, trace_cores=trace_cores
    )
    out = _assemble(res.results, np.asarray(inputs["b_out"]))
    return out, res


def kernel(x, grad, W_qkv, W_out, b_out):
    out, _ = _run(dict(x=x, grad=grad, W_qkv=W_qkv, W_out=W_out, b_out=b_out))
    return out
